# revision 1
# baseline (speedup 1.0000x reference)
"""DARTS mixed-op layer forward on 8 Trainium2 cores — fp16-pair matmuls.

Math: out[b,j] = sum_{i,k} softmax(alphas,axis=-1)[i,j,k] * coeffs[i,j,k] * prim_k(x[b,i])
with prims = [0, x, x^2, x^3, exp(x), ln(x), 1/x, sin(x)].  Channel 0 is zero, so
out = P @ W over 7 channels, W[(c,i),j] = gates[i,j,c+1]*coeffs[i,j,c+1] (softmax
denominator sums all 8 channels).

fp32 matmuls run at 4 cycles/column on the TRN2 PE; fp16 runs at 1.  Each fp32
value is split into an exact fp16 pair (hi = fp16(v), lo = fp16(v - hi), ~21
effective mantissa bits; the PE handles fp16 denormals exactly), and each
channel contraction becomes three fp16 matmuls: hi*Wh + lo*Wh + hi*Wl, which
recovers fp32-grade output accuracy (~1e-7 relative) at ~3/4 the PE cost of one
fp32 matmul... (3 cyc/col vs 4), and more importantly moves the elementwise
split work off the PE.

Sharding: batch split across 8 cores (8192 rows each).  The host uploads the
polynomial channels (x, x^2, x^3 — computed in fp32 exactly as the reference
does) pre-split into fp16 pairs in a paired-transpose layout
t[s, c*64+i, b] = T[s*256+c*128+b, i], so all elementwise work on device runs
with 128 SBUF partitions active.  The device computes exp/ln/recip/sin from the
reconstructed x, splits them, and contracts with block-diagonal duplicated
weights diag(W_c, W_c) so one K=128 matmul covers both 128-row batch chunks.
"""

import numpy as np

import concourse.bass as bass
import concourse.mybir as mybir
import concourse.tile as tile
from concourse import bacc
from concourse.bass_utils import run_bass_kernel_spmd

F32 = mybir.dt.float32
F16 = mybir.dt.float16
AFT = mybir.ActivationFunctionType

N_CORES = 8
BATCH = 65536
BC = BATCH // N_CORES          # 8192 rows per core
NCH = 7                        # nontrivial primitive channels


def build_kernel(bc: int = BC, repeat: int = 1) -> bass.Bass:
    nsup = bc // 256           # super-chunks of 256 rows
    fcols = nsup * 128         # paired-layout columns
    ng = fcols // 512          # matmul col-groups (PSUM banks used per pass)
    half = fcols // 2
    nseg = 4 if fcols % 2048 == 0 else 2
    seg = fcols // nseg

    nc = bacc.Bacc(None, target_bir_lowering=False, debug=False)
    xh_d = nc.dram_tensor("xh", [nsup, 128, 128], F16, kind="ExternalInput")
    xl_d = nc.dram_tensor("xl", [nsup, 128, 128], F16, kind="ExternalInput")
    sh_d = nc.dram_tensor("sh", [nsup, 128, 128], F16, kind="ExternalInput")
    sl_d = nc.dram_tensor("sl", [nsup, 128, 128], F16, kind="ExternalInput")
    ch_d = nc.dram_tensor("ch", [nsup, 128, 128], F16, kind="ExternalInput")
    cl_d = nc.dram_tensor("cl", [nsup, 128, 128], F16, kind="ExternalInput")
    aw = nc.dram_tensor("aw", [64, 512], F32, kind="ExternalInput")
    cw = nc.dram_tensor("cw", [64, 448], F32, kind="ExternalInput")
    ot = nc.dram_tensor("ot", [ng, 128, 512], F32, kind="ExternalOutput")

    with tile.TileContext(nc) as tc:
        import contextlib

        loop_ctx = tc.For_i(0, repeat, 1) if repeat > 1 else contextlib.nullcontext()
        with (
            loop_ctx,
            tc.tile_pool(name="pairs", bufs=1) as pairs,
            tc.tile_pool(name="big", bufs=1) as big,
            tc.tile_pool(name="scratch", bufs=2) as scratch,
            tc.tile_pool(name="small", bufs=1) as small,
            tc.tile_pool(name="outp", bufs=1) as outp,
            tc.tile_pool(name="psum", bufs=1, space="PSUM") as psum,
        ):
            # ---- gating inputs first: tiny, on the W critical path ----
            a8 = small.tile([64, 512], F32)
            nc.sync.dma_start(out=a8[:, :], in_=aw[:, :])
            c7 = small.tile([64, 448], F32)
            nc.sync.dma_start(out=c7[:, :], in_=cw[:, :])

            # ---- host-split channel pairs (paired layout) ----
            host_pairs = {}
            for idx, (name, dram) in enumerate(
                [("xh", xh_d), ("xl", xl_d), ("sh", sh_d),
                 ("sl", sl_d), ("ch", ch_d), ("cl", cl_d)]
            ):
                t = pairs.tile([128, fcols], F16, name=f"t_{name}")
                host_pairs[name] = t
                # xh/xl feed the xt32 critical path -> fast SP queue first;
                # sh/sl ride the idle ACT HWDGE; ch/cl on gpsimd SWDGE.
                eng = (nc.sync, nc.sync, nc.scalar,
                       nc.scalar, nc.sync, nc.scalar)[idx]
                eng.dma_start(
                    out=t.rearrange("p (s b) -> p s b", s=nsup),
                    in_=dram[:, :, :].rearrange("s p b -> p s b"),
                )

            # ---- gating: W[i,(c,j)] = exp(a)/sum_c8 exp(a) * coeffs ----
            e8 = small.tile([64, 512], F32)
            nc.scalar.activation(out=e8[:, :], in_=a8[:, :], func=AFT.Exp)
            s8 = small.tile([64, 64], F32)
            nc.vector.tensor_reduce(
                out=s8[:, :],
                in_=e8.rearrange("p (c j) -> p j c", c=8),
                axis=mybir.AxisListType.X,
                op=mybir.AluOpType.add,
            )
            r8 = small.tile([64, 64], F32)
            nc.vector.reciprocal(out=r8[:, :], in_=s8[:, :])
            w1 = small.tile([64, NCH, 64], F32)
            r8b = bass.AP(
                tensor=r8.tensor, offset=r8.offset, ap=[r8.ap[0], [0, NCH], [1, 64]]
            )
            nc.vector.tensor_mul(
                out=w1[:, :, :],
                in0=c7.rearrange("p (c j) -> p c j", c=NCH),
                in1=r8b,
            )
            wt = small.tile([64, NCH, 64], F32)
            nc.vector.tensor_mul(
                out=wt[:, :, :],
                in0=e8.rearrange("p (c j) -> p c j", c=8)[:, 1:8, :],
                in1=w1[:, :, :],
            )
            # fp16 split of the weights
            wh64 = small.tile([64, NCH, 64], F16)
            nc.vector.tensor_copy(out=wh64[:, :, :], in_=wt[:, :, :])
            wl64 = small.tile([64, NCH, 64], F16)
            nc.vector.tensor_sub(out=wl64[:, :, :], in0=wt[:, :, :], in1=wh64[:, :, :])
            # block-diagonal duplicates diag(W_c, W_c): one K=128 matmul covers
            # both 128-row batch chunks.  Partition-shifted copy via SBUF DMA.
            wtd_h = small.tile([128, NCH, 128], F16)
            wtd_l = small.tile([128, NCH, 128], F16)
            nc.vector.memset(wtd_h[:, :, :], 0.0)
            nc.vector.memset(wtd_l[:, :, :], 0.0)
            nc.vector.tensor_copy(out=wtd_h[0:64, :, 0:64], in_=wh64[:, :, :])
            nc.vector.tensor_copy(out=wtd_l[0:64, :, 0:64], in_=wl64[:, :, :])
            nc.sync.dma_start(out=wtd_h[64:128, :, 64:128], in_=wh64[:, :, :])
            nc.sync.dma_start(out=wtd_l[64:128, :, 64:128], in_=wl64[:, :, :])

            # ---- reconstruct x (fp32) for the transcendental channels ----
            xt32 = big.tile([128, fcols], F32)
            for h in range(nseg):
                c0, c1 = h * seg, (h + 1) * seg
                eng = nc.vector if h % 2 == 0 else nc.gpsimd
                eng.tensor_add(
                    out=xt32[:, c0:c1],
                    in0=host_pairs["xh"][:, c0:c1],
                    in1=host_pairs["xl"][:, c0:c1],
                )

            # ---- device channels: f32 -> fp16 pair ----
            dev_pairs = {}
            for name in ("ex", "lg", "rc", "sn"):
                dev_pairs[name] = (
                    big.tile([128, fcols], F16, name=f"{name}_hi"),
                    big.tile([128, fcols], F16, name=f"{name}_lo"),
                )

            def split_pair(name, f32src, h, cast_eng=None, sub_eng=None):
                hi, lo = dev_pairs[name]
                c0, c1 = h * seg, (h + 1) * seg
                (cast_eng or nc.vector).tensor_copy(out=hi[:, c0:c1], in_=f32src)
                (sub_eng or nc.vector).tensor_sub(
                    out=lo[:, c0:c1], in0=f32src, in1=hi[:, c0:c1]
                )

            # rc via fast reciprocal (51 ULP fp32 — well inside the error budget)
            for h in range(nseg):
                c0, c1 = h * seg, (h + 1) * seg
                rc32 = scratch.tile([128, seg], F32, name="rc32", tag="f32scratch")
                nc.vector.reciprocal_approx_fast(out=rc32[:, :], in_=xt32[:, c0:c1])
                split_pair("rc", rc32[:, :], h)
            for h in range(nseg):
                c0, c1 = h * seg, (h + 1) * seg
                ex32 = scratch.tile([128, seg], F32, name="ex32", tag="f32scratch")
                nc.scalar.activation(out=ex32[:, :], in_=xt32[:, c0:c1], func=AFT.Exp)
                split_pair("ex", ex32[:, :], h, cast_eng=nc.gpsimd, sub_eng=nc.gpsimd)
                lg32 = scratch.tile([128, seg], F32, name="lg32", tag="f32scratch")
                nc.scalar.activation(out=lg32[:, :], in_=xt32[:, c0:c1], func=AFT.Ln)
                split_pair("lg", lg32[:, :], h, cast_eng=nc.gpsimd)

            # ---- matmuls, channels except sin; sin appended after its ACT ----
            # order: host channels (DMA-ready) first; weights cycle per group.
            hp = host_pairs
            chan_pieces = [
                (hp["xh"], 0, "h"), (hp["xl"], 0, "h"), (hp["xh"], 0, "l"),
                (hp["sh"], 1, "h"), (hp["sl"], 1, "h"), (hp["sh"], 1, "l"),
                (hp["ch"], 2, "h"), (hp["cl"], 2, "h"), (hp["ch"], 2, "l"),
                (dev_pairs["ex"][0], 3, "h"), (dev_pairs["ex"][1], 3, "h"),
                (dev_pairs["ex"][0], 3, "l"),
                (dev_pairs["lg"][0], 4, "h"), (dev_pairs["lg"][1], 4, "h"),
                (dev_pairs["lg"][0], 4, "l"),
                (dev_pairs["rc"][0], 5, "h"), (dev_pairs["rc"][1], 5, "h"),
                (dev_pairs["rc"][0], 5, "l"),
            ]
            sin_pieces = [
                (dev_pairs["sn"][0], 6, "h"), (dev_pairs["sn"][1], 6, "h"),
                (dev_pairs["sn"][0], 6, "l"),
            ]

            ps = [psum.tile([128, 512], F32, name=f"ps{g}") for g in range(ng)]
            nblk = nseg if ng >= nseg else (2 if ng >= 2 else 1)
            gpb = ng // nblk  # groups per block (segment-aligned blocks)
            for blk in range(nblk):
                for pi, (data, ci, piece) in enumerate(chan_pieces):
                    w = wtd_h if piece == "h" else wtd_l
                    for g in range(blk * gpb, (blk + 1) * gpb):
                        nc.tensor.matmul(
                            ps[g][:, :],
                            w[:, ci, :],
                            data[:, g * 512:(g + 1) * 512],
                            start=(pi == 0),
                            stop=False,
                        )

            # ---- sin last (its ACT-table load happens once, after exp/ln) ----
            for h in range(nseg):
                c0, c1 = h * seg, (h + 1) * seg
                sn32 = scratch.tile([128, seg], F32, name="sn32", tag="f32scratch")
                nc.scalar.activation(out=sn32[:, :], in_=xt32[:, c0:c1], func=AFT.Sin)
                split_pair("sn", sn32[:, :], h)
            for blk in range(nblk):
                for pi, (data, ci, piece) in enumerate(sin_pieces):
                    w = wtd_h if piece == "h" else wtd_l
                    for g in range(blk * gpb, (blk + 1) * gpb):
                        nc.tensor.matmul(
                            ps[g][:, :],
                            w[:, ci, :],
                            data[:, g * 512:(g + 1) * 512],
                            start=False,
                            stop=(pi == len(sin_pieces) - 1),
                        )

            # ---- PSUM -> SBUF -> DRAM ----
            for g in range(ng):
                ob = outp.tile([128, 512], F32, name=f"ob{g}")
                nc.vector.tensor_copy(out=ob[:, :], in_=ps[g][:, :])
                nc.sync.dma_start(out=ot[g, :, :], in_=ob[:, :])

    nc.compile()
    return nc


_NC_CACHE: dict[int, bass.Bass] = {}


def _get_nc(bc: int = BC) -> bass.Bass:
    if bc not in _NC_CACHE:
        _NC_CACHE[bc] = build_kernel(bc)
    return _NC_CACHE[bc]


def _pair_layout(t: np.ndarray) -> np.ndarray:
    """[bc, 64] -> paired layout [nsup, 128, 128]: out[s, c*64+i, b] = t[s*256+c*128+b, i]."""
    nsup = t.shape[0] // 256
    return np.ascontiguousarray(
        t.reshape(nsup, 2, 128, 64).transpose(0, 1, 3, 2).reshape(nsup, 128, 128)
    )


def _split16(t: np.ndarray) -> tuple[np.ndarray, np.ndarray]:
    hi = t.astype(np.float16)
    lo = (t.astype(np.float64) - hi.astype(np.float64)).astype(np.float16)
    return hi, lo


def _prep_shard(xs: np.ndarray) -> dict[str, np.ndarray]:
    xs = xs.astype(np.float32)
    sq = xs * xs                      # fp32, bit-identical to the reference
    cu = sq * xs
    out = {}
    for name, t in [("x", xs), ("s", sq), ("c", cu)]:
        hi, lo = _split16(t)
        out[name + "h"] = _pair_layout(hi)
        out[name + "l"] = _pair_layout(lo)
    return out


def _unshard_out(ot: np.ndarray) -> np.ndarray:
    ng = ot.shape[0]
    return (
        ot.reshape(ng, 2, 64, 4, 128)
        .transpose(0, 3, 1, 4, 2)
        .reshape(ng * 1024, 64)
    )


def kernel(x: np.ndarray, alphas: np.ndarray, coeffs: np.ndarray) -> np.ndarray:
    x = np.asarray(x, dtype=np.float32)
    alphas = np.asarray(alphas, dtype=np.float32)
    coeffs = np.asarray(coeffs, dtype=np.float32)

    aw = np.ascontiguousarray(alphas.transpose(0, 2, 1).reshape(64, 512))
    cw = np.ascontiguousarray(coeffs[:, :, 1:].transpose(0, 2, 1).reshape(64, 448))

    bc = x.shape[0] // N_CORES
    in_maps = []
    for c in range(N_CORES):
        m = _prep_shard(x[c * bc:(c + 1) * bc])
        m["aw"] = aw
        m["cw"] = cw
        in_maps.append(m)

    nc = _get_nc(bc)
    res = run_bass_kernel_spmd(nc, in_maps, core_ids=list(range(N_CORES)))
    return np.concatenate([_unshard_out(r["ot"]) for r in res.results], axis=0)



# revision 3
# speedup vs baseline: 19.1479x; 19.1479x over previous
"""DARTS mixed-op layer forward on 8 Trainium2 cores — cubic-fold fp16 matmuls.

Math: out[b,j] = sum_{i,k} softmax(alphas,axis=-1)[i,j,k] * coeffs[i,j,k] * prim_k(x[b,i])
with prims = [0, x, x^2, x^3, exp(x), ln(x), 1/x, sin(x)] and x in (0.5, 1.5).

Key restructure: on (0.5, 1.5) each transcendental primitive is replaced by a
least-squares cubic, so the whole per-(i,j) mixture collapses to a single cubic
    f_ij(x) = C0[i,j] + C1[i,j] x + C2[i,j] x^2 + C3[i,j] x^3
folded ON THE HOST from gates*coeffs and the fixed poly coefficients (end-to-end
max-rel error ~3.5e-3 incl. fp16, vs the 2e-2 gate).  The device then only does
    out[b,j] = bias[j] + sum_i sum_{p=1..3} Cp[i,j] * x_i^p
i.e. THREE fp16 matmul channels (x, x^2, x^3) instead of the reference's seven
transcendental channels: 12288 PE rows/core at 1 cyc/row fp16.

Sharding: batch split across 8 cores (8192 rows each).  Paired layout packs two
128-row batch chunks into the 128 SBUF partitions (p = c*64 + i) and the weights
are block-diagonal diag(Cp, Cp), so one K=128 matmul covers both chunks (1.5 PE
cycles per batch row — the K<=128 floor for a 192-wide contraction).  x^2/x^3
are fp16 DVE muls on device (2x mode); PSUM is evicted by ACT with the bias add
fused (Identity activation + per-partition bias) straight to fp16 output.
"""

import numpy as np

import concourse.bass as bass
import concourse.mybir as mybir
import concourse.tile as tile
from concourse import bacc
from concourse.bass_utils import run_bass_kernel_spmd

F32 = mybir.dt.float32
F16 = mybir.dt.float16
AFT = mybir.ActivationFunctionType

N_CORES = 8
BATCH = 65536
BC = BATCH // N_CORES          # 8192 rows per core
DEG = 3                        # cubic fold

# least-squares cubic fits of the transcendental prims on (0.5, 1.5),
# computed once at import time (input-independent).
def _poly_fits(deg: int = DEG) -> dict[int, np.ndarray]:
    grid = np.linspace(0.5, 1.5, 20001)
    V = np.vander(grid, deg + 1, increasing=True)
    fits = {}
    for k, f in ((4, np.exp), (5, np.log), (6, lambda t: 1.0 / t), (7, np.sin)):
        fits[k] = np.linalg.lstsq(V, f(grid), rcond=None)[0]
    return fits

_FITS = _poly_fits()


def build_kernel(bc: int = BC, repeat: int = 1) -> bass.Bass:
    fcols = bc // 2            # paired-layout columns (2 batch rows per col)
    ng = fcols // 512          # PSUM banks / matmul col-groups
    gblk = 2                   # groups per PE ordering block

    nc = bacc.Bacc(None, target_bir_lowering=False, debug=False)
    xh_d = nc.dram_tensor("xh", [128, fcols], F16, kind="ExternalInput")
    wd_d = nc.dram_tensor("wd", [128, DEG * 128], F16, kind="ExternalInput")
    bs_d = nc.dram_tensor("bs", [128, 1], F32, kind="ExternalInput")
    ot_d = nc.dram_tensor("ot", [128, fcols], F16, kind="ExternalOutput")

    with tile.TileContext(nc) as tc:
        import contextlib

        loop_ctx = tc.For_i(0, repeat, 1) if repeat > 1 else contextlib.nullcontext()
        with (
            loop_ctx,
            tc.tile_pool(name="big", bufs=1) as big,
            tc.tile_pool(name="small", bufs=1) as small,
            tc.tile_pool(name="outp", bufs=1) as outp,
            tc.tile_pool(name="psum", bufs=1, space="PSUM") as psum,
        ):
            # ---- weights + bias on the SP queue (critical path head) ----
            wd = small.tile([128, DEG, 128], F16)
            nc.sync.dma_start(out=wd[:, :, :],
                              in_=wd_d[:, :].rearrange("p (c j) -> p c j", c=DEG))
            bs = small.tile([128, 1], F32)
            nc.sync.dma_start(out=bs[:, :], in_=bs_d[:, :])

            # ---- x groups: even on ACT queue (starts sooner), odd on SP ----
            xt = big.tile([128, fcols], F16, name="xt")
            for g in range(ng):
                eng = nc.scalar if g % 2 == 0 else nc.sync
                c0, c1 = g * 512, (g + 1) * 512
                eng.dma_start(out=xt[:, c0:c1], in_=xh_d[:, c0:c1])

            x2 = big.tile([128, fcols], F16, name="x2")
            x3 = big.tile([128, fcols], F16, name="x3")
            ps = [psum.tile([128, 512], F32, name=f"ps{g}") for g in range(ng)]

            def muls(g):
                c0, c1 = g * 512, (g + 1) * 512
                nc.vector.tensor_mul(out=x2[:, c0:c1], in0=xt[:, c0:c1],
                                     in1=xt[:, c0:c1])
                nc.vector.tensor_mul(out=x3[:, c0:c1], in0=x2[:, c0:c1],
                                     in1=xt[:, c0:c1])

            def mm(g, p, data, start, stop):
                c0, c1 = g * 512, (g + 1) * 512
                nc.tensor.matmul(ps[g][:, :], wd[:, p, :], data[:, c0:c1],
                                 start=start, stop=stop)

            def evict(g):
                c0, c1 = g * 512, (g + 1) * 512
                ob = outp.tile([128, fcols], F16, name="ob")
                nc.scalar.activation(out=ob[:, c0:c1], in_=ps[g][:, :],
                                     func=AFT.Identity, bias=bs[:, 0:1])
                eng = nc.sync if g % 2 == 0 else nc.gpsimd
                eng.dma_start(out=ot_d[:, c0:c1], in_=ob[:, c0:c1])

            # DVE muls stream ahead in group order
            for g in range(ng):
                muls(g)

            # PE: blocks of gblk groups, channel-major inside a block so the
            # stationary weight switches only DEG times per block; banks of a
            # finished block are evicted while the next block computes.
            for b0 in range(0, ng, gblk):
                gs = range(b0, min(b0 + gblk, ng))
                for p, data in ((0, xt), (1, x2), (2, x3)):
                    for g in gs:
                        mm(g, p, data, start=(p == 0), stop=(p == DEG - 1))
                for g in gs:
                    evict(g)

    nc.compile()
    return nc


_NC_CACHE: dict[int, bass.Bass] = {}


def _get_nc(bc: int = BC) -> bass.Bass:
    if bc not in _NC_CACHE:
        _NC_CACHE[bc] = build_kernel(bc)
    return _NC_CACHE[bc]


def _pair_layout(t: np.ndarray) -> np.ndarray:
    """[bc, 64] fp16 -> [128, bc/2]: out[c*64+i, s*128+b] = t[s*256+c*128+b, i]."""
    nsup = t.shape[0] // 256
    # [nsup, 2, 128, 64] -> (c, i) on partitions, (s, b) on free
    return np.ascontiguousarray(
        t.reshape(nsup, 2, 128, 64).transpose(1, 3, 0, 2).reshape(128, nsup * 128)
    )


def _fold_weights(alphas: np.ndarray, coeffs: np.ndarray):
    """Fold gates*coeffs and the cubic fits into C[p][i,j] (p=0..3)."""
    a = alphas.astype(np.float64)
    e = np.exp(a - a.max(-1, keepdims=True))
    gates = e / e.sum(-1, keepdims=True)
    w = gates * coeffs.astype(np.float64)              # [I, J, K]
    C = np.zeros((DEG + 1, 64, 64))
    for p in (1, 2, 3):                                # exact power channels
        C[p] += w[:, :, p]
    for k, fit in _FITS.items():                       # folded transcendentals
        for p in range(DEG + 1):
            C[p] += w[:, :, k] * fit[p]
    return C


def kernel(x: np.ndarray, alphas: np.ndarray, coeffs: np.ndarray) -> np.ndarray:
    x = np.asarray(x, dtype=np.float32)
    C = _fold_weights(np.asarray(alphas), np.asarray(coeffs))

    # block-diagonal duplicated weights diag(Cp, Cp), fp16
    wd = np.zeros((128, DEG, 128), np.float16)
    for p in (1, 2, 3):
        wd[0:64, p - 1, 0:64] = C[p].astype(np.float16)
        wd[64:128, p - 1, 64:128] = C[p].astype(np.float16)
    wd = wd.reshape(128, DEG * 128)
    bias = np.tile(C[0].sum(0).astype(np.float32), 2).reshape(128, 1)

    bc = x.shape[0] // N_CORES
    in_maps = []
    for c in range(N_CORES):
        xs = x[c * bc:(c + 1) * bc].astype(np.float16)
        in_maps.append({"xh": _pair_layout(xs), "wd": wd, "bs": bias})

    nc = _get_nc(bc)
    res = run_bass_kernel_spmd(nc, in_maps, core_ids=list(range(N_CORES)))

    outs = []
    for r in res.results:
        ot = r["ot"].astype(np.float32)                # [128, bc/2]
        nsup = bc // 256
        # ot[c*64+j, s*128+b] -> out[s*256+c*128+b, j]
        outs.append(
            ot.reshape(2, 64, nsup, 128).transpose(2, 0, 3, 1).reshape(bc, 64)
        )
    return np.concatenate(outs, axis=0)
